# revision 10
# baseline (speedup 1.0000x reference)
"""LIF neuron kernel for Trainium2 (Bass/Tile), 8-core SPMD.

Reference computation (per problem nn_LIF_69707319214329):
    v_new      = v * DECAY + sum(x, axis=1) * 10         # [IN]
    fired      = v_new >= THRESHOLD                      # [IN]
    spikes_new = where(fired, 1.0, spikes)               # [IN]
    out        = spikes_new[None, :] * weight            # [OUT, IN]

Sharding: in_features (columns of weight / rows of x) are split into 8
contiguous blocks of 1024.  Core j receives x rows [1024j, 1024j+1024),
the matching v/spikes slices, and weight[:, block] (made contiguous on the
host).  Each core computes its own spikes slice locally -- no collectives --
and produces out[:, block].  Per-core HBM traffic: 4MB x + 32MB weight read
+ 32MB output write (68MB -> ~199us at the 358GB/s HBM-per-core limit).

Schedule (v2): the kernel is DMA-bound, so both HWDGE rings must stream
from t=0 and the store stream must start as early as possible.
  * Phase 1 (LIF state) is off the weight-load path: x is loaded as 4 x 1MB
    tiles alternating rings, the threshold comparison operands (thresh =
    2 - v*DECAY/10, ones, spk=spikes copy) are precomputed as soon as their
    tiny inputs land, and the spike-row flatten DMA is issued via SWDGE
    (gpsimd) so it never sits ahead of weight loads on a FIFO HWDGE ring.
  * Phase 2 is graded: the first weight segments are small (1MB/2MB) so the
    first multiply -- and therefore the ring-serialized store stream --
    starts ~6us earlier than with uniform 4MB tiles.
  * Ring balance: ACT loads the first two (small) weight segments and
    stores the rest; sync loads the rest and stores one early-computed
    segment at the end.  Per pass each ring moves exactly 32MB.
"""

import math

import numpy as np

import concourse.bass as bass
import concourse.bacc as bacc
import concourse.mybir as mybir
from concourse.tile import TileContext
from concourse.bass_utils import run_bass_kernel_spmd

N_CORES = 8
IN_FEATURES = 8192
OUT_FEATURES = 8192
K = 1024
SHARD = IN_FEATURES // N_CORES          # 1024 in_features per core
TAU = 1.0
THRESHOLD = 20.0
DECAY = math.exp(-0.01 / TAU)

F32 = mybir.dt.float32

# Main-loop tiling: weight shard [8192, 1024] seen as segments of
# [128, rpp * 1024]; partition p of a segment starting at row0 holds weight
# rows row0 + p*rpp ... row0 + p*rpp + rpp - 1 (contiguous bytes).
T_COLS = SHARD // 128                   # 8 state columns

# PE_REDUCE: the host supplies x TRANSPOSED ([K, SHARD]); the row-sum over K
# becomes a partition-dim reduction done by the (otherwise idle) tensor
# engine via ones[128,1].T @ xT_tile, accumulated in PSUM -- the spike row is
# born as [1, SHARD] on partition 0, so the [128,8]->[1,1024] flatten DMA and
# the host row-permutation disappear, and all LIF elementwise ops run on
# plain [1, SHARD] rows.  Measured in CoreSim: LOSES to the DVE path (24.3us
# vs 11.1us non-pass overhead) -- fp32 matmul is quarter-rate on PE (853ns
# per 512 cols, 16 arrival-gated matmuls = 13.7us serial) and the [1, SHARD]
# v/s row loads are priced like full-ring DMAs.  Kept as a documented dead
# end; default off.
PE_REDUCE = False
X_TILES = SHARD // 128                  # 8 xT-tiles of 0.5MB ([128, K] rows)
MM_N = 512                              # PE moving-tensor free-dim limit

# non-PE fallback: x in natural layout, host-permuted so the LIF state tile
# is rs[p, c] = local in_feature 8p + c (flatten == identity iteration).
X_ROWS_PER_PART = 2
_J = np.arange(SHARD)
PERM = 8 * (_J % 128) + _J // 128

DEF_PATTERN = (2, 2, 4) + (8,) * 7      # rows-of-128 per segment; sum = 64
DEF_ACT_W = (0, 1)                      # segments whose weight load is on ACT
DEF_SYNC_ST = (2,)                      # segments whose store is on sync


def _build_bass(
    reps: int = 1,
    pattern: tuple = DEF_PATTERN,
    act_w_segs: tuple = DEF_ACT_W,
    sync_st_segs: tuple = DEF_SYNC_ST,
    wbufs: int = 4,
    overlap: bool = True,
    fake_spikes: bool = False,
) -> bass.Bass:
    """reps>1 repeats the phase-2 weight stream (for HW timing via deltas);
    output is identical since every pass writes the same values."""
    assert sum(pattern) * 128 == OUT_FEATURES
    segments = []          # (row_start, rows_per_part)
    row0 = 0
    for rpp in pattern:
        segments.append((row0, rpp))
        row0 += 128 * rpp

    nc = bacc.Bacc(
        "TRN2",
        target_bir_lowering=False,
        debug=False,
        num_devices=N_CORES,
    )

    if PE_REDUCE:
        x = nc.dram_tensor("x", [K, SHARD], F32, kind="ExternalInput")
        v = nc.dram_tensor("v", [1, SHARD], F32, kind="ExternalInput")
        s = nc.dram_tensor("s", [1, SHARD], F32, kind="ExternalInput")
    else:
        x = nc.dram_tensor("x", [SHARD, K], F32, kind="ExternalInput")
        v = nc.dram_tensor("v", [128, T_COLS], F32, kind="ExternalInput")
        s = nc.dram_tensor("s", [128, T_COLS], F32, kind="ExternalInput")
    w = nc.dram_tensor("w", [OUT_FEATURES, SHARD], F32, kind="ExternalInput")
    o = nc.dram_tensor("o", [OUT_FEATURES, SHARD], F32, kind="ExternalOutput")

    with TileContext(nc) as tc:
        with (
            tc.tile_pool(name="state", bufs=1) as state,
            tc.tile_pool(
                name="xp",
                bufs=X_TILES if PE_REDUCE else SHARD // (128 * X_ROWS_PER_PART),
            ) as xp,
            tc.tile_pool(name="wp", bufs=wbufs) as wp,
            tc.psum_pool(name="pp", bufs=1) as pp,
        ):
            # ---- Phase 1: LIF state -> broadcast spike row ----
            if fake_spikes:
                # timing-only variant: skip the LIF state computation to
                # measure phase-1's marginal cost (inputs left unread)
                bb = state.tile([128, SHARD], F32)
                nc.vector.memset(bb[:], 1.0)
            elif PE_REDUCE:
                srow = [1, SHARD]
                vt = state.tile(srow, F32)
                st = state.tile(srow, F32)
                nc.sync.dma_start(out=vt[:], in_=v[:])
                nc.sync.dma_start(out=st[:], in_=s[:])

                # operands with no dependency on the reduction -- computed
                # while x is still streaming.  fired = (v*DECAY + rs*10 >=
                # 20)  <=>  rs >= 2 - v*(DECAY/10).
                ones_col = state.tile([128, 1], F32)
                nc.vector.memset(ones_col[:], 1.0)
                thresh = state.tile(srow, F32)
                nc.vector.tensor_scalar(
                    out=thresh[:],
                    in0=vt[:],
                    scalar1=-(DECAY / 10.0),
                    scalar2=2.0,
                    op0=mybir.AluOpType.mult,
                    op1=mybir.AluOpType.add,
                )
                ones_row = state.tile(srow, F32)
                nc.vector.memset(ones_row[:], 1.0)
                spk = state.tile(srow, F32)
                nc.vector.tensor_copy(out=spk[:], in_=st[:])

                # rs[0, i] = sum_k x[k, i] via ones[128,1].T @ xT_tile,
                # accumulated across the 8 K-chunks in PSUM.
                ps = pp.tile([1, SHARD], F32)
                for t in range(X_TILES):
                    xt = xp.tile([128, SHARD], F32)
                    dma_eng = nc.sync if t % 2 == 0 else nc.scalar
                    dma_eng.dma_start(out=xt[:], in_=x[t * 128:(t + 1) * 128, :])
                    for h in range(SHARD // MM_N):
                        nc.tensor.matmul(
                            out=ps[:1, h * MM_N:(h + 1) * MM_N],
                            lhsT=ones_col[:, :1],
                            rhs=xt[:, h * MM_N:(h + 1) * MM_N],
                            start=(t == 0),
                            stop=(t == X_TILES - 1),
                        )

                rs = state.tile(srow, F32)
                nc.vector.tensor_copy(out=rs[:], in_=ps[:])
                mask = state.tile(srow, mybir.dt.uint32)
                nc.vector.tensor_tensor(
                    out=mask[:],
                    in0=rs[:],
                    in1=thresh[:],
                    op=mybir.AluOpType.is_ge,
                )
                nc.vector.copy_predicated(spk[:], mask[:], ones_row[:])

                # broadcast the spike row to all partitions
                bb = state.tile([128, SHARD], F32)
                nc.gpsimd.partition_broadcast(bb[:], spk[:1, :])
            else:
                rs = state.tile([128, T_COLS], F32)
                for t in range(SHARD // (128 * X_ROWS_PER_PART)):
                    a = X_ROWS_PER_PART
                    xt = xp.tile([128, a, K], F32)
                    src = x[t * 128 * a:(t + 1) * 128 * a, :]
                    src = src.rearrange("(a p) c -> p a c", p=128)
                    # split x loads across both HWDGE rings (SP + ACT)
                    dma_eng = nc.sync if t % 2 == 0 else nc.scalar
                    dma_eng.dma_start(out=xt[:], in_=src)
                    nc.vector.reduce_sum(
                        out=rs[:, t * a:(t + 1) * a],
                        in_=xt[:],
                        axis=mybir.AxisListType.X,
                    )

                vt = state.tile([128, T_COLS], F32)
                st = state.tile([128, T_COLS], F32)
                nc.sync.dma_start(out=vt[:], in_=v[:])
                nc.sync.dma_start(out=st[:], in_=s[:])

                spk = state.tile([128, T_COLS], F32)
                ones = state.tile([128, T_COLS], F32)
                # fired = (v*DECAY + rs*10 >= 20)  <=>  rs >= 2 - v*(DECAY/10)
                thresh = state.tile([128, T_COLS], F32)
                nc.vector.tensor_scalar(
                    out=thresh[:],
                    in0=vt[:],
                    scalar1=-(DECAY / 10.0),
                    scalar2=2.0,
                    op0=mybir.AluOpType.mult,
                    op1=mybir.AluOpType.add,
                )
                nc.vector.memset(ones[:], 1.0)
                nc.vector.tensor_copy(out=spk[:], in_=st[:])
                mask = state.tile([128, T_COLS], mybir.dt.uint32)
                nc.vector.tensor_tensor(
                    out=mask[:],
                    in0=rs[:],
                    in1=thresh[:],
                    op=mybir.AluOpType.is_ge,
                )
                nc.vector.copy_predicated(spk[:], mask[:], ones[:])

                # flatten spk [128, T_COLS] -> row [1, SHARD] (identity
                # iteration thanks to the host permutation), via SWDGE so
                # this compute-dependent DMA never sits ahead of phase-2
                # weight loads on a FIFO HWDGE ring.
                row = state.tile([1, SHARD], F32)
                flat_eng = nc.gpsimd if overlap else nc.sync
                flat_eng.dma_start(out=row[:1, :], in_=spk[:])

                # broadcast the spike row to all partitions
                bb = state.tile([128, SHARD], F32)
                nc.gpsimd.partition_broadcast(bb[:], row[:1, :])

            bb_row = bb[:, :].rearrange("p (z c) -> p z c", z=1)
            bb_bcast = {
                rpp: bb_row.broadcast_to([128, rpp, SHARD])
                for rpp in set(pattern)
            }

            # ---- Phase 2: out = weight * spikes (column-broadcast) ----
            for i, (row0, rpp) in enumerate(
                sg for _ in range(reps) for sg in segments
            ):
                seg = i % len(segments)
                ld_eng = nc.scalar if seg in act_w_segs else nc.sync
                st_eng = nc.sync if seg in sync_st_segs else nc.scalar
                nrows = 128 * rpp
                wt = wp.tile([128, rpp * SHARD], F32, tag="wt")
                src = w[row0:row0 + nrows, :]
                src = src.rearrange("(p a) c -> p (a c)", a=rpp)
                ld_eng.dma_start(out=wt[:], in_=src)

                nc.vector.tensor_mul(
                    out=wt[:].rearrange("p (a c) -> p a c", a=rpp),
                    in0=wt[:].rearrange("p (a c) -> p a c", a=rpp),
                    in1=bb_bcast[rpp],
                )

                dst = o[row0:row0 + nrows, :]
                dst = dst.rearrange("(p a) c -> p (a c)", a=rpp)
                st_eng.dma_start(out=dst, in_=wt[:])

    nc.compile()
    return nc


_NC_CACHE = {}


def _get_bass(reps: int = 1, **kwargs) -> bass.Bass:
    key = (reps, tuple(sorted(kwargs.items())))
    if key not in _NC_CACHE:
        _NC_CACHE[key] = _build_bass(reps, **kwargs)
    return _NC_CACHE[key]


def _shard_inputs(x, weight, v, spikes):
    in_maps = []
    for j in range(N_CORES):
        sl = slice(j * SHARD, (j + 1) * SHARD)
        if PE_REDUCE:
            in_maps.append({
                "x": np.ascontiguousarray(x[sl, :].T),
                "w": np.ascontiguousarray(weight[:, sl]),
                "v": np.ascontiguousarray(v[sl].reshape(1, SHARD)),
                "s": np.ascontiguousarray(spikes[sl].reshape(1, SHARD)),
            })
        else:
            in_maps.append({
                "x": np.ascontiguousarray(x[sl, :][PERM]),
                "w": np.ascontiguousarray(weight[:, sl]),
                "v": np.ascontiguousarray(v[sl].reshape(128, T_COLS)),
                "s": np.ascontiguousarray(spikes[sl].reshape(128, T_COLS)),
            })
    return in_maps


def run(x, weight, v, spikes, trace=False, **run_kwargs):
    """Run the 8-core kernel; returns (full_output, BassKernelResults)."""
    x = np.asarray(x, dtype=np.float32)
    weight = np.asarray(weight, dtype=np.float32)
    v = np.asarray(v, dtype=np.float32)
    spikes = np.asarray(spikes, dtype=np.float32)
    assert x.shape == (IN_FEATURES, K)
    assert weight.shape == (OUT_FEATURES, IN_FEATURES)

    nc = _get_bass()
    in_maps = _shard_inputs(x, weight, v, spikes)
    res = run_bass_kernel_spmd(
        nc, in_maps, core_ids=list(range(N_CORES)), trace=trace, **run_kwargs
    )
    out = np.empty((OUT_FEATURES, IN_FEATURES), dtype=np.float32)
    for j in range(N_CORES):
        out[:, j * SHARD:(j + 1) * SHARD] = res.results[j]["o"]
    return out, res


def kernel(x, weight, v, spikes, t=None, **_ignored):
    out, _ = run(x, weight, v, spikes, trace=False)
    return out


# revision 14
# speedup vs baseline: 1.0259x; 1.0259x over previous
"""LIF neuron kernel for Trainium2 (Bass/Tile), 8-core SPMD.

Reference computation (per problem nn_LIF_69707319214329):
    v_new      = v * DECAY + sum(x, axis=1) * 10         # [IN]
    fired      = v_new >= THRESHOLD                      # [IN]
    spikes_new = where(fired, 1.0, spikes)               # [IN]
    out        = spikes_new[None, :] * weight            # [OUT, IN]

Sharding: in_features (columns of weight / rows of x) are split into 8
contiguous blocks of 1024.  Core j receives x rows [1024j, 1024j+1024),
the matching v/spikes slices, and weight[:, block] (made contiguous on the
host).  Each core computes its own spikes slice locally -- no collectives --
and produces out[:, block].  Per-core HBM traffic: 4MB x + 32MB weight read
+ 32MB output write (68MB -> ~199us at the 358GB/s HBM-per-core limit).

Schedule (v2): the kernel is DMA-bound, so both HWDGE rings must stream
from t=0 and the store stream must start as early as possible.
  * Phase 1 (LIF state) is off the weight-load path: x is loaded as 4 x 1MB
    tiles alternating rings, the threshold comparison operands (thresh =
    2 - v*DECAY/10, ones, spk=spikes copy) are precomputed as soon as their
    tiny inputs land, and the spike-row flatten DMA is issued via SWDGE
    (gpsimd) so it never sits ahead of weight loads on a FIFO HWDGE ring.
  * Phase 2 is graded: the first weight segments are small (1MB/2MB) so the
    first multiply -- and therefore the ring-serialized store stream --
    starts ~6us earlier than with uniform 4MB tiles.
  * Ring balance: ACT loads the first two (small) weight segments and
    stores the rest; sync loads the rest and stores one early-computed
    segment at the end.  Per pass each ring moves exactly 32MB.
"""

import math

import numpy as np

import concourse.bass as bass
import concourse.bacc as bacc
import concourse.mybir as mybir
from concourse.tile import TileContext
from concourse.bass_utils import run_bass_kernel_spmd

N_CORES = 8
IN_FEATURES = 8192
OUT_FEATURES = 8192
K = 1024
SHARD = IN_FEATURES // N_CORES          # 1024 in_features per core
TAU = 1.0
THRESHOLD = 20.0
DECAY = math.exp(-0.01 / TAU)

F32 = mybir.dt.float32

# Main-loop tiling: weight shard [8192, 1024] seen as segments of
# [128, rpp * 1024]; partition p of a segment starting at row0 holds weight
# rows row0 + p*rpp ... row0 + p*rpp + rpp - 1 (contiguous bytes).
T_COLS = SHARD // 128                   # 8 state columns

# PE_REDUCE: the host supplies x TRANSPOSED ([K, SHARD]); the row-sum over K
# becomes a partition-dim reduction done by the (otherwise idle) tensor
# engine via ones[128,1].T @ xT_tile, accumulated in PSUM -- the spike row is
# born as [1, SHARD] on partition 0, so the [128,8]->[1,1024] flatten DMA and
# the host row-permutation disappear, and all LIF elementwise ops run on
# plain [1, SHARD] rows.  Measured in CoreSim: LOSES to the DVE path (24.3us
# vs 11.1us non-pass overhead) -- fp32 matmul is quarter-rate on PE (853ns
# per 512 cols, 16 arrival-gated matmuls = 13.7us serial) and the [1, SHARD]
# v/s row loads are priced like full-ring DMAs.  Kept as a documented dead
# end; default off.
PE_REDUCE = False
X_TILES = SHARD // 128                  # 8 xT-tiles of 0.5MB ([128, K] rows)
MM_N = 512                              # PE moving-tensor free-dim limit

# non-PE fallback: x in natural layout, host-permuted so the LIF state tile
# is rs[p, c] = local in_feature 8p + c (flatten == identity iteration).
X_ROWS_PER_PART = 2
_J = np.arange(SHARD)
PERM = 8 * (_J % 128) + _J // 128

DEF_PATTERN = (2, 2, 4) + (8,) * 7      # rows-of-128 per segment; sum = 64
DEF_ACT_W = (0, 1)                      # segments whose weight load is on ACT
DEF_SYNC_ST = (2,)                      # segments whose store is on sync


def _build_bass(
    reps: int = 1,
    pattern: tuple = DEF_PATTERN,
    act_w_segs: tuple = DEF_ACT_W,
    sync_st_segs: tuple = DEF_SYNC_ST,
    wbufs: int = 4,
    overlap: bool = True,
    fake_spikes: bool = False,
) -> bass.Bass:
    """reps>1 repeats the phase-2 weight stream (for HW timing via deltas);
    output is identical since every pass writes the same values."""
    assert sum(pattern) * 128 == OUT_FEATURES
    segments = []          # (row_start, rows_per_part)
    row0 = 0
    for rpp in pattern:
        segments.append((row0, rpp))
        row0 += 128 * rpp

    nc = bacc.Bacc(
        "TRN2",
        target_bir_lowering=False,
        debug=False,
        num_devices=N_CORES,
    )

    if PE_REDUCE:
        x = nc.dram_tensor("x", [K, SHARD], F32, kind="ExternalInput")
        v = nc.dram_tensor("v", [1, SHARD], F32, kind="ExternalInput")
        s = nc.dram_tensor("s", [1, SHARD], F32, kind="ExternalInput")
    else:
        x = nc.dram_tensor("x", [SHARD, K], F32, kind="ExternalInput")
        v = nc.dram_tensor("v", [128, T_COLS], F32, kind="ExternalInput")
        s = nc.dram_tensor("s", [128, T_COLS], F32, kind="ExternalInput")
    w = nc.dram_tensor("w", [OUT_FEATURES, SHARD], F32, kind="ExternalInput")
    o = nc.dram_tensor("o", [OUT_FEATURES, SHARD], F32, kind="ExternalOutput")

    with TileContext(nc) as tc:
        with (
            tc.tile_pool(name="state", bufs=1) as state,
            tc.tile_pool(
                name="xp",
                bufs=X_TILES if PE_REDUCE else SHARD // (128 * X_ROWS_PER_PART),
            ) as xp,
            tc.tile_pool(name="wp", bufs=wbufs) as wp,
            tc.psum_pool(name="pp", bufs=1) as pp,
        ):
            # ---- Phase 1: LIF state -> broadcast spike row ----
            if fake_spikes:
                # timing-only variant: skip the LIF state computation to
                # measure phase-1's marginal cost (inputs left unread)
                bb = state.tile([128, SHARD], F32)
                nc.vector.memset(bb[:], 1.0)
            elif PE_REDUCE:
                srow = [1, SHARD]
                vt = state.tile(srow, F32)
                st = state.tile(srow, F32)
                nc.sync.dma_start(out=vt[:], in_=v[:])
                nc.sync.dma_start(out=st[:], in_=s[:])

                # operands with no dependency on the reduction -- computed
                # while x is still streaming.  fired = (v*DECAY + rs*10 >=
                # 20)  <=>  rs >= 2 - v*(DECAY/10).
                ones_col = state.tile([128, 1], F32)
                nc.vector.memset(ones_col[:], 1.0)
                thresh = state.tile(srow, F32)
                nc.vector.tensor_scalar(
                    out=thresh[:],
                    in0=vt[:],
                    scalar1=-(DECAY / 10.0),
                    scalar2=2.0,
                    op0=mybir.AluOpType.mult,
                    op1=mybir.AluOpType.add,
                )
                ones_row = state.tile(srow, F32)
                nc.vector.memset(ones_row[:], 1.0)
                spk = state.tile(srow, F32)
                nc.vector.tensor_copy(out=spk[:], in_=st[:])

                # rs[0, i] = sum_k x[k, i] via ones[128,1].T @ xT_tile,
                # accumulated across the 8 K-chunks in PSUM.
                ps = pp.tile([1, SHARD], F32)
                for t in range(X_TILES):
                    xt = xp.tile([128, SHARD], F32)
                    dma_eng = nc.sync if t % 2 == 0 else nc.scalar
                    dma_eng.dma_start(out=xt[:], in_=x[t * 128:(t + 1) * 128, :])
                    for h in range(SHARD // MM_N):
                        nc.tensor.matmul(
                            out=ps[:1, h * MM_N:(h + 1) * MM_N],
                            lhsT=ones_col[:, :1],
                            rhs=xt[:, h * MM_N:(h + 1) * MM_N],
                            start=(t == 0),
                            stop=(t == X_TILES - 1),
                        )

                rs = state.tile(srow, F32)
                nc.vector.tensor_copy(out=rs[:], in_=ps[:])
                mask = state.tile(srow, mybir.dt.uint32)
                nc.vector.tensor_tensor(
                    out=mask[:],
                    in0=rs[:],
                    in1=thresh[:],
                    op=mybir.AluOpType.is_ge,
                )
                nc.vector.copy_predicated(spk[:], mask[:], ones_row[:])

                # broadcast the spike row to all partitions
                bb = state.tile([128, SHARD], F32)
                nc.gpsimd.partition_broadcast(bb[:], spk[:1, :])
            else:
                # NOTE: splitting the reduction across engines (DVE tiles
                # 0-1, ACT activation-accum tiles 2-3) was tried and LOSES
                # in the cost model (overhead 15.2us vs 11.1us): the ACT
                # compute displaces ACT's early weight-load issues, delaying
                # the store stream more than the shorter reduce tail saves.
                rs = state.tile([128, T_COLS], F32)
                for t in range(SHARD // (128 * X_ROWS_PER_PART)):
                    a = X_ROWS_PER_PART
                    xt = xp.tile([128, a, K], F32)
                    src = x[t * 128 * a:(t + 1) * 128 * a, :]
                    src = src.rearrange("(a p) c -> p a c", p=128)
                    # split x loads across both HWDGE rings (SP + ACT)
                    dma_eng = nc.sync if t % 2 == 0 else nc.scalar
                    dma_eng.dma_start(out=xt[:], in_=src)
                    nc.vector.reduce_sum(
                        out=rs[:, t * a:(t + 1) * a],
                        in_=xt[:],
                        axis=mybir.AxisListType.X,
                    )

                vt = state.tile([128, T_COLS], F32)
                st = state.tile([128, T_COLS], F32)
                nc.sync.dma_start(out=vt[:], in_=v[:])
                nc.sync.dma_start(out=st[:], in_=s[:])

                spk = state.tile([128, T_COLS], F32)
                ones = state.tile([128, T_COLS], F32)
                # fired = (v*DECAY + rs*10 >= 20)  <=>  rs >= 2 - v*(DECAY/10)
                thresh = state.tile([128, T_COLS], F32)
                nc.vector.tensor_scalar(
                    out=thresh[:],
                    in0=vt[:],
                    scalar1=-(DECAY / 10.0),
                    scalar2=2.0,
                    op0=mybir.AluOpType.mult,
                    op1=mybir.AluOpType.add,
                )
                nc.vector.memset(ones[:], 1.0)
                nc.vector.tensor_copy(out=spk[:], in_=st[:])
                mask = state.tile([128, T_COLS], mybir.dt.uint32)
                nc.vector.tensor_tensor(
                    out=mask[:],
                    in0=rs[:],
                    in1=thresh[:],
                    op=mybir.AluOpType.is_ge,
                )
                nc.vector.copy_predicated(spk[:], mask[:], ones[:])

                # flatten spk [128, T_COLS] -> row [1, SHARD] (identity
                # iteration thanks to the host permutation), via SWDGE so
                # this compute-dependent DMA never sits ahead of phase-2
                # weight loads on a FIFO HWDGE ring.  (Splitting it into
                # two 64-descriptor halves on gpsimd+sync was tried: worse,
                # 13.0us vs 11.1us overhead in the cost model.)
                row = state.tile([1, SHARD], F32)
                flat_eng = nc.gpsimd if overlap else nc.sync
                flat_eng.dma_start(out=row[:1, :], in_=spk[:])

                # broadcast the spike row to all partitions
                bb = state.tile([128, SHARD], F32)
                nc.gpsimd.partition_broadcast(bb[:], row[:1, :])

            bb_row = bb[:, :].rearrange("p (z c) -> p z c", z=1)
            bb_bcast = {
                rpp: bb_row.broadcast_to([128, rpp, SHARD])
                for rpp in set(pattern)
            }

            # ---- Phase 2: out = weight * spikes (column-broadcast) ----
            for i, (row0, rpp) in enumerate(
                sg for _ in range(reps) for sg in segments
            ):
                seg = i % len(segments)
                ld_eng = nc.scalar if seg in act_w_segs else nc.sync
                st_eng = nc.sync if seg in sync_st_segs else nc.scalar
                nrows = 128 * rpp
                wt = wp.tile([128, rpp * SHARD], F32, tag="wt")
                src = w[row0:row0 + nrows, :]
                src = src.rearrange("(p a) c -> p (a c)", a=rpp)
                ld_eng.dma_start(out=wt[:], in_=src)

                nc.vector.tensor_mul(
                    out=wt[:].rearrange("p (a c) -> p a c", a=rpp),
                    in0=wt[:].rearrange("p (a c) -> p a c", a=rpp),
                    in1=bb_bcast[rpp],
                )

                dst = o[row0:row0 + nrows, :]
                dst = dst.rearrange("(p a) c -> p (a c)", a=rpp)
                st_eng.dma_start(out=dst, in_=wt[:])

    nc.compile()
    return nc


_NC_CACHE = {}


def _get_bass(reps: int = 1, **kwargs) -> bass.Bass:
    key = (reps, tuple(sorted(kwargs.items())))
    if key not in _NC_CACHE:
        _NC_CACHE[key] = _build_bass(reps, **kwargs)
    return _NC_CACHE[key]


def _shard_inputs(x, weight, v, spikes):
    in_maps = []
    for j in range(N_CORES):
        sl = slice(j * SHARD, (j + 1) * SHARD)
        if PE_REDUCE:
            in_maps.append({
                "x": np.ascontiguousarray(x[sl, :].T),
                "w": np.ascontiguousarray(weight[:, sl]),
                "v": np.ascontiguousarray(v[sl].reshape(1, SHARD)),
                "s": np.ascontiguousarray(spikes[sl].reshape(1, SHARD)),
            })
        else:
            in_maps.append({
                "x": np.ascontiguousarray(x[sl, :][PERM]),
                "w": np.ascontiguousarray(weight[:, sl]),
                "v": np.ascontiguousarray(v[sl].reshape(128, T_COLS)),
                "s": np.ascontiguousarray(spikes[sl].reshape(128, T_COLS)),
            })
    return in_maps


def run(x, weight, v, spikes, trace=False, **run_kwargs):
    """Run the 8-core kernel; returns (full_output, BassKernelResults)."""
    x = np.asarray(x, dtype=np.float32)
    weight = np.asarray(weight, dtype=np.float32)
    v = np.asarray(v, dtype=np.float32)
    spikes = np.asarray(spikes, dtype=np.float32)
    assert x.shape == (IN_FEATURES, K)
    assert weight.shape == (OUT_FEATURES, IN_FEATURES)

    nc = _get_bass()
    in_maps = _shard_inputs(x, weight, v, spikes)
    res = run_bass_kernel_spmd(
        nc, in_maps, core_ids=list(range(N_CORES)), trace=trace, **run_kwargs
    )
    out = np.empty((OUT_FEATURES, IN_FEATURES), dtype=np.float32)
    for j in range(N_CORES):
        out[:, j * SHARD:(j + 1) * SHARD] = res.results[j]["o"]
    return out, res


def kernel(x, weight, v, spikes, t=None, **_ignored):
    out, _ = run(x, weight, v, spikes, trace=False)
    return out
